# revision 2
# baseline (speedup 1.0000x reference)
"""CrossAttention2D Trainium2 kernel (v2: bf16 + O-orientation attention).

Sharding: data-parallel over batch. B=8 -> one batch element per NeuronCore,
no collectives. Weights replicated; host pre-transposes and casts to bf16.

Per-core math (C=512, Ccross=768, N=1024, 8 heads x 64):
  Q = Wq @ x_b          [C, N]   bf16
  K = Wk @ y_b          [C, N]   bf16
  VTa = [(Wv @ y_b).T | 1]       [N, 8*(64+1)] bf16 (ones col per head)
  per head pair ph (heads at PE rows 0/64, row-tiled scores):
    S[k, q] = K_h^T Q_h          psum [128, 1024] per (kt, half), ping-pong
    ET = exp(S/8)                ACT -> SBUF bf16, resident for whole pair
    O_aug[q, 0:65] = ET_tile^T @ VTa_h   (ET stationary: 65-col streams,
                                          accumulated over kt; col 64 = denom)
    O = O_aug[:, :64] / O_aug[:, 64]     (DVE reciprocal + scale) -> bf16
  quirk: out_flat[h*64 + q//16, 64*(q%16) + d] = O_h[q, d]  (DMA shuffle)
  out = Wo @ quirk + bo          [C, N] -> bf16 out, host casts to f32
"""

import numpy as np
import ml_dtypes

import concourse.bass as bass
import concourse.mybir as mybir
import concourse.tile as tile
from concourse import bacc
from concourse.bass_utils import run_bass_kernel_spmd

P = 128
C = 512          # d_embed
CC = 768         # d_cross
N = 1024         # H*W = 32*32
NH = 8
DH = 64
CT = C // P      # 4
CCT = CC // P    # 6
QT = N // P      # 8
HW = 32
B = 8
F32 = mybir.dt.float32
BF16 = mybir.dt.bfloat16

_CACHE = {}


def _build_nc():
    nc = bacc.Bacc("TRN2", target_bir_lowering=False, debug=False, num_devices=B)

    x = nc.dram_tensor("x", [C, N], BF16, kind="ExternalInput")
    y = nc.dram_tensor("y", [CC, N], BF16, kind="ExternalInput")
    wqT = nc.dram_tensor("wqT", [C, C], BF16, kind="ExternalInput")
    wkT = nc.dram_tensor("wkT", [CC, C], BF16, kind="ExternalInput")
    wvT = nc.dram_tensor("wvT", [CC, C], BF16, kind="ExternalInput")
    woT = nc.dram_tensor("woT", [C, C], BF16, kind="ExternalInput")
    bq = nc.dram_tensor("bq", [C], F32, kind="ExternalInput")
    bk = nc.dram_tensor("bk", [C], F32, kind="ExternalInput")
    bv = nc.dram_tensor("bv", [C], BF16, kind="ExternalInput")
    bo = nc.dram_tensor("bo", [C], F32, kind="ExternalInput")
    out = nc.dram_tensor("out", [C, N], BF16, kind="ExternalOutput")

    EXP = mybir.ActivationFunctionType.Exp

    with tile.TileContext(nc) as tc:
        with (
            tc.tile_pool(name="const", bufs=1) as constp,
            tc.tile_pool(name="big", bufs=1) as bigp,
            tc.tile_pool(name="et", bufs=16) as etp,
            tc.tile_pool(name="oa", bufs=3) as oap,
            tc.tile_pool(name="ev", bufs=3) as evp,
            tc.tile_pool(name="rcp", bufs=4) as rcpp,
            tc.tile_pool(name="psS", bufs=2, space="PSUM") as psS,
            tc.tile_pool(name="psAV", bufs=1, space="PSUM") as psAV,
            tc.tile_pool(name="psP", bufs=2, space="PSUM") as psP,
        ):
            # ---- constants ----
            ones_r = constp.tile([1, P], BF16, name="ones_r", tag="ones_r")
            nc.vector.memset(ones_r[:], 1.0)
            # preload the exp table set early so the ~2.7us ACT_TABLE_LOAD
            # overlaps the input DMA phase instead of the first real exp
            dmy = constp.tile([P, 1], F32, name="dmy", tag="dmy")
            nc.vector.memset(dmy[:], 0.0)
            dmy2 = constp.tile([P, 1], F32, name="dmy2", tag="dmy2")
            nc.scalar.activation(dmy2[:], dmy[:], EXP)

            bq_sb = constp.tile([P, CT], F32, name="bq", tag="bq")
            nc.sync.dma_start(bq_sb[:], bq.rearrange("(o p) -> p o", p=P))
            bk_sb = constp.tile([P, CT], F32, name="bk", tag="bk")
            nc.sync.dma_start(bk_sb[:], bk.rearrange("(o p) -> p o", p=P))
            bo_sb = constp.tile([P, CT], F32, name="bo", tag="bo")
            nc.sync.dma_start(bo_sb[:], bo.rearrange("(o p) -> p o", p=P))
            bv_sb = constp.tile([1, C], BF16, name="bv", tag="bv")
            nc.sync.dma_start(bv_sb[:], bv[None, :])

            # ---- weight / activation loads (per-ktile DMAs for queue spread) ----
            x3 = x.rearrange("(t p) n -> p t n", p=P)
            y3 = y.rearrange("(t p) n -> p t n", p=P)
            wq3 = wqT.rearrange("(t p) m -> p t m", p=P)
            wk3 = wkT.rearrange("(t p) m -> p t m", p=P)
            wv3 = wvT.rearrange("(t p) m -> p t m", p=P)
            wo3 = woT.rearrange("(t p) m -> p t m", p=P)

            x_sb = [bigp.tile([P, N], BF16, name=f"x{t}", tag=f"x{t}") for t in range(CT)]
            y_sb = [bigp.tile([P, N], BF16, name=f"y{t}", tag=f"y{t}") for t in range(CCT)]
            wq_sb = [bigp.tile([P, C], BF16, name=f"wq{t}", tag=f"wq{t}") for t in range(CT)]
            wk_sb = [bigp.tile([P, C], BF16, name=f"wk{t}", tag=f"wk{t}") for t in range(CCT)]
            wv_sb = [bigp.tile([P, C], BF16, name=f"wv{t}", tag=f"wv{t}") for t in range(CCT)]
            wo_sb = [bigp.tile([P, C], BF16, name=f"wo{t}", tag=f"wo{t}") for t in range(CT)]
            for t in range(CT):
                nc.sync.dma_start(x_sb[t][:], x3[:, t])
                nc.sync.dma_start(wq_sb[t][:], wq3[:, t])
            for t in range(CCT):
                nc.sync.dma_start(y_sb[t][:], y3[:, t])
                nc.sync.dma_start(wk_sb[t][:], wk3[:, t])
            for t in range(CCT):
                nc.sync.dma_start(wv_sb[t][:], wv3[:, t])
            for t in range(CT):
                nc.sync.dma_start(wo_sb[t][:], wo3[:, t])

            q_sb = [bigp.tile([P, N], BF16, name=f"q{t}", tag=f"q{t}") for t in range(CT)]
            k_sb = [bigp.tile([P, N], BF16, name=f"k{t}", tag=f"k{t}") for t in range(CT)]
            # VTa buffer: per n-tile, cols laid out [h][65] with col h*65+64 == 1.0
            vt_sb = [bigp.tile([P, NH * (DH + 1)], BF16, name=f"vt{t}", tag=f"vt{t}")
                     for t in range(QT)]
            for t in range(QT):
                nc.gpsimd.memset(vt_sb[t][:], 1.0)

            # ---- projection helpers ----
            def qk_proj(ct, dst, w_tiles, src_tiles, nkt, bias_sb):
                for half in range(2):
                    ps = psP.tile([P, 512], F32, name="ps", tag="psp")
                    for kt in range(nkt):
                        nc.tensor.matmul(
                            ps[:],
                            w_tiles[kt][:, ct * P:(ct + 1) * P],
                            src_tiles[kt][:, half * 512:(half + 1) * 512],
                            start=(kt == 0),
                            stop=(kt == nkt - 1),
                        )
                    nc.vector.tensor_scalar_add(
                        dst[:, half * 512:(half + 1) * 512], ps[:], bias_sb[:, ct:ct + 1]
                    )

            # ---- VT projection: VT[n, c] = sum_k y[k, n] * wvT[k, c]  (+ bias row)
            def vt_proj(nt):
                ps = psP.tile([P, 512], F32, name="ps", tag="psp")
                for kt in range(CCT):
                    nc.tensor.matmul(
                        ps[:],
                        y_sb[kt][:, nt * P:(nt + 1) * P],
                        wv_sb[kt][:],
                        start=(kt == 0),
                        stop=False,
                    )
                nc.tensor.matmul(ps[:], ones_r[:], bv_sb[:], start=False, stop=True)
                # scatter into [h][0:64] slots (col h*65+64 stays 1.0)
                nc.vector.tensor_copy(
                    out=vt_sb[nt].rearrange("p (h e) -> p h e", e=DH + 1)[:, :, 0:DH],
                    in_=ps.rearrange("p (h d) -> p h d", d=DH),
                )

            qk_proj(0, q_sb[0], wq_sb, x_sb, CT, bq_sb)
            qk_proj(0, k_sb[0], wk_sb, y_sb, CCT, bk_sb)
            for nt in range(QT):
                vt_proj(nt)

            # ---- attention: per pair, scores+exp; AV trails via resident ET ----
            qk_sb = [bigp.tile([P, N], BF16, name=f"qk{t}", tag=f"qk{t}")
                     for t in range(CT)]

            def emit_scores(ph):
                """Scores + exp for both heads of pair ph; ET tiles kept in SBUF."""
                ets = []
                for kt in range(QT):
                    et = etp.tile([P, 2048], BF16, name="et", tag="et")
                    ets.append(et)
                    for half in range(2):
                        sps = psS.tile([P, 1024], F32, name="sps", tag="pss")
                        for hh in range(2):
                            bp = hh * DH
                            nc.tensor.matmul(
                                sps[:, hh * 512:(hh + 1) * 512],
                                k_sb[ph][bp:bp + DH, kt * P:(kt + 1) * P],
                                q_sb[ph][bp:bp + DH, half * 512:(half + 1) * 512],
                                start=True,
                                stop=True,
                            )
                        nc.scalar.activation(
                            et[:, half * 1024:(half + 1) * 1024], sps[:],
                            EXP, scale=0.125,
                        )
                return ets

            def emit_av(ph, ets):
                """O = softmax(S) @ V^T for both heads of pair ph, via resident ET."""
                for hh in range(2):
                    h = 2 * ph + hh
                    av = psAV.tile([P, 1024], F32, name="av", tag="av")
                    oa = oap.tile([P, 512], BF16, name="oa", tag="oa")
                    for qt in range(QT):
                        ecol = (qt // 4) * 1024 + hh * 512 + (qt % 4) * P
                        for kt in range(QT):
                            nc.tensor.matmul(
                                av[:, qt * P:qt * P + DH + 1],
                                ets[kt][:, ecol:ecol + P],
                                vt_sb[kt][:, h * (DH + 1):(h + 1) * (DH + 1)],
                                start=(kt == 0),
                                stop=(kt == QT - 1),
                            )
                        rcp = rcpp.tile([P, 1], F32, name="rcp", tag="rcp")
                        nc.vector.reciprocal(
                            rcp[:], av[:, qt * P + DH:qt * P + DH + 1]
                        )
                        nc.vector.tensor_scalar_mul(
                            oa[:, qt * DH:(qt + 1) * DH],
                            av[:, qt * P:qt * P + DH], rcp[:],
                        )
                        # quirk shuffle: qk[ph][hh*64 + qt*8 + p//16, 64*(p%16)+d]
                        #   = O_h[qt*128 + p, d]
                        nc.sync.dma_start(
                            qk_sb[ph][hh * 64 + qt * 8: hh * 64 + qt * 8 + 8, :],
                            oa[:, qt * DH:(qt + 1) * DH],
                        )

            prev = None
            for ph in range(NH // 2):
                ets = emit_scores(ph)
                if ph + 1 < NH // 2:
                    qk_proj(ph + 1, q_sb[ph + 1], wq_sb, x_sb, CT, bq_sb)
                    qk_proj(ph + 1, k_sb[ph + 1], wk_sb, y_sb, CCT, bk_sb)
                if prev is not None:
                    emit_av(ph - 1, prev)
                prev = ets
            emit_av(NH // 2 - 1, prev)

            # ---- output projection ----
            out3 = out.rearrange("(t p) n -> p t n", p=P)
            for ct in range(CT):
                for half in range(2):
                    ps = psP.tile([P, 512], F32, name="ps", tag="psp")
                    for kt in range(CT):
                        nc.tensor.matmul(
                            ps[:],
                            wo_sb[kt][:, ct * P:(ct + 1) * P],
                            qk_sb[kt][:, half * 512:(half + 1) * 512],
                            start=(kt == 0),
                            stop=(kt == CT - 1),
                        )
                    ev = evp.tile([P, 512], BF16, name="ev", tag="ev")
                    nc.vector.tensor_scalar_add(ev[:], ps[:], bo_sb[:, ct:ct + 1])
                    nc.sync.dma_start(out3[:, ct, half * 512:(half + 1) * 512], ev[:])

    nc.compile()
    return nc


def kernel(**inputs) -> np.ndarray:
    bf = ml_dtypes.bfloat16
    x = np.ascontiguousarray(np.asarray(inputs["x"], dtype=np.float32).astype(bf))
    y = np.ascontiguousarray(np.asarray(inputs["y"], dtype=np.float32).astype(bf))
    wqT = np.ascontiguousarray(np.asarray(inputs["w_q"], dtype=np.float32).T.astype(bf))
    wkT = np.ascontiguousarray(np.asarray(inputs["w_k"], dtype=np.float32).T.astype(bf))
    wvT = np.ascontiguousarray(np.asarray(inputs["w_v"], dtype=np.float32).T.astype(bf))
    woT = np.ascontiguousarray(np.asarray(inputs["w_o"], dtype=np.float32).T.astype(bf))
    bq = np.ascontiguousarray(np.asarray(inputs["b_q"], dtype=np.float32))
    bk = np.ascontiguousarray(np.asarray(inputs["b_k"], dtype=np.float32))
    bv = np.ascontiguousarray(np.asarray(inputs["b_v"], dtype=np.float32).astype(bf))
    bo = np.ascontiguousarray(np.asarray(inputs["b_o"], dtype=np.float32))

    if "nc" not in _CACHE:
        _CACHE["nc"] = _build_nc()
    nc = _CACHE["nc"]

    in_maps = []
    for b in range(B):
        in_maps.append({
            "x": np.ascontiguousarray(x[b].reshape(C, N)),
            "y": np.ascontiguousarray(y[b].reshape(CC, N)),
            "wqT": wqT, "wkT": wkT, "wvT": wvT, "woT": woT,
            "bq": bq, "bk": bk, "bv": bv, "bo": bo,
        })
    res = run_bass_kernel_spmd(nc, in_maps, core_ids=list(range(B)))
    return np.stack([
        np.asarray(res.results[b]["out"]).astype(np.float32).reshape(C, HW, HW)
        for b in range(B)
    ])
